# revision 28
# baseline (speedup 1.0000x reference)
"""Trainium2 Bass kernel for the three-GEU (text/video/audio) embedding model.

Strategy (8 NeuronCores, zero collectives):
  - Algebraic fusion on host: g = h @ Wg^T + bg with h = x @ W^T + b collapses
    to g = x @ (Wg W)^T + (Wg b + bg).  The gating GEMM then reads the SAME
    pooled activations x as the first GEMM, so no h AllGather is needed.  The
    audio gating weight also shrinks from 4096x4096 to 4096x1024.
  - Pooling (text max over L, audio ragged masked-mean) and the final L2
    normalization are O(B*D) host work; the device only runs the GEMM stack.
  - Tensor-parallel column sharding: core c owns output columns
    [512c, 512(c+1)) of every linear.  It streams an 18 MiB fp16 "weight
    wall" (all six W^T slices, k-tile major, in consumption order) in chunks
    alternating across the two HWDGE queues, and consumes them with
    acts-stationary matmuls (out = xT_tile.T @ w_tile, N=512).  A small
    first chunk plus a split xT DMA gets real matmuls started by ~12us, and
    a burst of junk matmuls bridges the preamble so the PE HAM clock is
    already at 2.4 GHz when the stream begins.
  - The last GEMM (audio gating) is split into column halves so its
    sigmoid/mul epilogue overlaps the second half's matmuls; y ships fp16.
    Host gathers the 8 column shards, L2-normalizes, returns fp32.
"""

import numpy as np

B = 64
L = 30
D = 4096
DA = 1024
T = 128
NCORES = 8
S = D // NCORES        # 512: per-core output shard of D
KT_D = D // 128        # 32 k-tiles over D
KT_A = DA // 128       # 8 k-tiles over Da
KT_X = 2 * KT_D + KT_A  # 72 k-tiles of pooled acts (text, video, audio)

# weight wall, flat fp16 column space per partition, consumption order:
#   text-h(32kt x 512), text-g(32x512), video-h(32x512), video-g(32x512),
#   audio gating halves agA/agB (8x256 each), then audio-h halves ahA/ahB
#   LAST so the sigmoids hide under the h matmuls and the kernel tail is
#   just (8 matmuls + mul + out-DMA) per half.
GEMM_BASE = {"th": 0, "tg": 16384, "vh": 32768, "vg": 49152,
             "agA": 65536, "agB": 67584, "ahA": 69632, "ahB": 71680}
WALL_COLS = 73728
# uniform 1 MiB chunks: arrival cadence ~2.4us interleaved across the two
# queues keeps the PE fed in-order (2 MiB chunks starved it for ~5us per
# queue rotation); 0.5 MiB tail chunks minimize the last-matmul tail
CH_COLS = [4096] * 17 + [2048] * 2                  # 19 chunks
CH_CUM = np.cumsum([0] + CH_COLS).tolist()
N_JUNK = 12                                          # PE warm-up matmuls
EMBEDS = ("text", "video", "audio")

_STATE: dict = {}


def _build():
    from contextlib import ExitStack

    import concourse.bass as bass  # noqa: F401
    import concourse.tile as tile
    from concourse import bacc, mybir

    fp16 = mybir.dt.float16
    f32 = mybir.dt.float32
    ACTF = mybir.ActivationFunctionType

    nc = bacc.Bacc(
        "TRN2",
        target_bir_lowering=False,
        debug=False,
        enable_asserts=False,
        num_devices=NCORES,
    )

    # chunk-contiguous packing: chunk ch occupies the flat byte range
    # [128*CH_CUM[ch], 128*CH_CUM[ch+1]) so each chunk DMA is one fully
    # contiguous HBM read (strided reads measured ~12% slower).
    wall_d = nc.dram_tensor("wall", [1, 128 * WALL_COLS], fp16,
                            kind="ExternalInput")
    xT_d = nc.dram_tensor("xT", [128, KT_X * B], fp16, kind="ExternalInput")
    bias_d = nc.dram_tensor("biases", [1, 6 * S], fp16, kind="ExternalInput")
    out_d = {e: nc.dram_tensor(f"out_{e}", [B, S], fp16,
                               kind="ExternalOutput")
             for e in EMBEDS}

    with ExitStack() as ctx:
        tc = ctx.enter_context(tile.TileContext(nc))

        persist = ctx.enter_context(tc.tile_pool(name="persist", bufs=1))
        # bufs=16: all but the last 3 chunk DMAs pre-issue with no
        # consumption coupling — per-ring FIFO keeps head-of-line chunks
        # completing at full queue rate, so a deep pre-loaded ring gives a
        # guaranteed ~2.4us completion cadence with zero trigger latency.
        wpool = ctx.enter_context(tc.tile_pool(name="wstream", bufs=16))
        work = ctx.enter_context(tc.tile_pool(name="work", bufs=2))
        psum = ctx.enter_context(tc.tile_pool(name="psum", bufs=4,
                                              space="PSUM"))
        jpool = ctx.enter_context(tc.tile_pool(name="jpsum", bufs=1,
                                               space="PSUM"))

        xT = persist.tile([128, KT_X, B], fp16)
        bias_sb = persist.tile([1, 6, S], fp16)
        ones_sb = persist.tile([1, B], fp16)
        warm = persist.tile([64, 2], f32)
        z_sb = persist.tile([128, 576], fp16)

        # constants off the DMA queues
        nc.vector.memset(z_sb[:], 0.0)
        nc.vector.memset(ones_sb[:], 1.0)
        nc.vector.memset(warm[:], 0.0)

        # activation DMAs on the scalar HWDGE ring: text xT first so the
        # first k-matmuls can start as soon as chunk 0 lands; the
        # video+audio part of xT rides between chunks 3 and 5 (it isn't
        # needed until chunk 6) to keep chunk 1 near the front of the ring.
        xTv = xT.rearrange("p k b -> p (k b)")
        nc.scalar.dma_start(bias_sb.rearrange("p s x -> p (s x)"),
                            bias_d.ap())
        nc.scalar.dma_start(xTv[:, 0:KT_D * B], xT_d.ap()[:, 0:KT_D * B])
        # ACT sigmoid table pre-load AFTER the scalar DMA triggers (the
        # 1.5us ACT_TABLE_LOAD would otherwise delay the whole stream)
        nc.scalar.activation(warm[:, 0:1], warm[:, 1:2], ACTF.Sigmoid)

        # junk matmuls: keep the PE busy from the preamble until chunk 0
        # arrives so HAM un-throttles to 2.4 GHz before the real stream.
        junk_ps = jpool.tile([B, S], f32)
        for _ in range(N_JUNK):
            nc.tensor.matmul(junk_ps[:], z_sb[:, 0:B], z_sb[:, B:B + S],
                             start=True, stop=True)

        # weight wall chunk stream, alternating HWDGE queues in order.
        # Each chunk's DMA is chained on the COMPLETION of the chunk 4
        # earlier (same queue): the rings fair-share bandwidth at packet
        # granularity, so a deep trigger window only inflates every
        # chunk's completion latency (8 MiB in flight measured ~19us to
        # the gating semaphore), while consumption-paced triggers leave
        # the rings under-filled.  Completion-paced depth-4 keeps one
        # transfer running + one queued per ring: full rate, in order.
        hwdge = [nc.sync, nc.scalar]
        wtiles = []
        for ch in range(len(CH_COLS)):
            w = wpool.tile([128, CH_COLS[ch]], fp16, name="wch", tag="wch")
            hwdge[ch % 2].dma_start(
                w[:],
                wall_d.ap()[0, 128 * CH_CUM[ch]:128 * CH_CUM[ch + 1]]
                .rearrange("(p c) -> p c", p=128))
            wtiles.append(w)
            if ch == 3:
                nc.scalar.dma_start(xTv[:, KT_D * B:],
                                    xT_d.ap()[:, KT_D * B:])

        def wchunk(gemm, kt, width=512):
            c = GEMM_BASE[gemm] + kt * width
            ch = 0
            while CH_CUM[ch + 1] <= c:
                ch += 1
            return ch, c - CH_CUM[ch]

        ps = {}
        last_ch = [0]

        def gemm(tag, bias_ap, xbase, nkt, width=512):
            p = psum.tile([B, width], f32, name=f"ps_{tag}", tag="ps")
            ps[tag] = p
            nc.tensor.matmul(p[:], ones_sb[:], bias_ap, start=True,
                             stop=False)
            for k in range(nkt):
                ch, off = wchunk(tag, k, width)
                if ch != last_ch[0] and ch < 10:
                    # HAM insurance at each early chunk boundary: if the
                    # next chunk is late these keep the PE busy so the
                    # clock never re-throttles to 1.2 GHz (which would
                    # cascade: slow matmuls -> stalled buffer releases ->
                    # stalled weight DMAs).
                    for _ in range(2):
                        nc.tensor.matmul(junk_ps[:], z_sb[:, 0:B],
                                         z_sb[:, B:B + S],
                                         start=True, stop=True)
                last_ch[0] = ch
                nc.tensor.matmul(p[:], xT[:, xbase + k, :],
                                 wtiles[ch][:, off:off + width],
                                 start=False, stop=(k == nkt - 1))
            return p

        # text / video: full-width h and g GEMMs + epilogue
        gemm("th", bias_sb[:, 0, :], 0, KT_D)
        gemm("tg", bias_sb[:, 1, :], 0, KT_D)
        sg = work.tile([B, S], f32, name="sg", tag="sg")
        nc.scalar.activation(sg[:], ps["tg"][:], ACTF.Sigmoid)
        y_t = work.tile([B, S], fp16, name="y", tag="y")
        nc.vector.tensor_mul(y_t[:], ps["th"][:], sg[:])

        gemm("vh", bias_sb[:, 2, :], KT_D, KT_D)
        gemm("vg", bias_sb[:, 3, :], KT_D, KT_D)
        sg = work.tile([B, S], f32, name="sg", tag="sg")
        nc.scalar.activation(sg[:], ps["vg"][:], ACTF.Sigmoid)
        y_v = work.tile([B, S], fp16, name="y", tag="y")
        nc.vector.tensor_mul(y_v[:], ps["vh"][:], sg[:])

        # audio, all in column halves: gating first (sigmoids hide under
        # the h matmuls), h halves last so the kernel tail is minimal
        gemm("agA", bias_sb[:, 5, 0:256], 2 * KT_D, KT_A, width=256)
        sgA = work.tile([B, 256], f32, name="sg", tag="sg")
        nc.scalar.activation(sgA[:], ps["agA"][:], ACTF.Sigmoid)
        gemm("agB", bias_sb[:, 5, 256:512], 2 * KT_D, KT_A, width=256)
        sgB = work.tile([B, 256], f32, name="sg", tag="sg")
        nc.scalar.activation(sgB[:], ps["agB"][:], ACTF.Sigmoid)
        gemm("ahA", bias_sb[:, 4, 0:256], 2 * KT_D, KT_A, width=256)
        y_a0 = work.tile([B, 256], fp16, name="y", tag="y")
        nc.vector.tensor_mul(y_a0[:], ps["ahA"][:], sgA[:])
        gemm("ahB", bias_sb[:, 4, 256:512], 2 * KT_D, KT_A, width=256)
        y_a1 = work.tile([B, 256], fp16, name="y", tag="y")
        nc.vector.tensor_mul(y_a1[:], ps["ahB"][:], sgB[:])

        # output DMAs, emitted after every chunk trigger on each queue so
        # they never block the weight stream; audio halves land last.
        nc.sync.dma_start(out_d["text"].ap(), y_t[:])
        nc.scalar.dma_start(out_d["video"].ap(), y_v[:])
        nc.sync.dma_start(out_d["audio"].ap()[:, 0:256], y_a0[:])
        nc.scalar.dma_start(out_d["audio"].ap()[:, 256:512], y_a1[:])

    nc.compile()
    return nc


def _get_nc():
    if "nc" not in _STATE:
        _STATE["nc"] = _build()
    return _STATE["nc"]


def _fuse_weights(Wt, bt, Wgt, bgt, Wv, bv, Wgv, bgv, Wa, ba, Wga, bga):
    """Fold each gating linear through its fc linear; shard into walls."""
    key = tuple(id(a) for a in (Wt, Wgt, Wv, Wgv, Wa, Wga))
    cached = _STATE.get("fused")
    if cached is not None and cached[0] == key:
        return cached[1], cached[2]

    f16 = np.float16
    Ws = [np.asarray(w, np.float32) for w in (Wt, Wgt, Wv, Wgv, Wa, Wga)]
    bs = [np.asarray(b, np.float32) for b in (bt, bgt, bv, bgv, ba, bga)]
    Wt, Wgt, Wv, Wgv, Wa, Wga = Ws
    bt, bgt, bv, bgv, ba, bga = bs

    Wgt_f = Wgt @ Wt
    bgt_f = Wgt @ bt + bgt
    Wgv_f = Wgv @ Wv
    bgv_f = Wgv @ bv + bgv
    Wga_f = Wga @ Wa
    bga_f = Wga @ ba + bga

    walls, biases = [], []
    for c in range(NCORES):
        sl = slice(c * S, (c + 1) * S)
        cols = []
        for M in (Wt, Wgt_f, Wv, Wgv_f):
            Mt = M[sl, :].T                              # [K, S]
            nkt = Mt.shape[0] // 128
            cols.append(Mt.reshape(nkt, 128, S)
                        .transpose(1, 0, 2).reshape(128, nkt * S))
        Ga = Wga_f[sl, :].T                              # [1024, 512]
        Ha = Wa[sl, :].T                                 # [1024, 512]
        for half in (Ga[:, 0:256], Ga[:, 256:512],
                     Ha[:, 0:256], Ha[:, 256:512]):
            cols.append(np.ascontiguousarray(half)
                        .reshape(KT_A, 128, 256)
                        .transpose(1, 0, 2).reshape(128, KT_A * 256))
        wall = np.ascontiguousarray(np.concatenate(cols, axis=1)).astype(f16)
        assert wall.shape == (128, WALL_COLS)
        # pack chunk-contiguous: chunk ch = wall[:, c0:c1] flattened p-major
        flat = np.concatenate(
            [wall[:, CH_CUM[ch]:CH_CUM[ch + 1]].reshape(-1)
             for ch in range(len(CH_COLS))])
        walls.append(flat.reshape(1, -1))
        biases.append(np.stack([bt[sl], bgt_f[sl], bv[sl], bgv_f[sl],
                                ba[sl], bga_f[sl]])
                      .reshape(1, -1).astype(f16))
    _STATE["fused"] = (key, walls, biases)
    _STATE["fused_refs"] = (Ws, bs)   # keep ids alive for the cache key
    return walls, biases


def _prep_in_maps(text, video, audio_feats, Wt, bt, Wgt, bgt, Wv, bv,
                  Wgv, bgv, Wa, ba, Wga, bga, nframes, raw_audio_len):
    f16 = np.float16
    text = np.asarray(text, np.float32)
    video = np.asarray(video, np.float32)
    audio = np.asarray(audio_feats, np.float32)

    # host pooling: text max over L; audio ragged masked mean over T
    pooled_text = text.max(axis=1)                                  # [B, D]
    ratio = int(round(float(np.asarray(raw_audio_len)) / T))
    nf = np.maximum(
        1, (np.asarray(nframes).astype(np.float32) / ratio).astype(np.int32))
    mask = (np.arange(T)[None, :] < nf[:, None]).astype(np.float32)
    pooled_audio = np.einsum('bct,bt->bc', audio, mask) / nf[:, None]

    xT = np.concatenate([pooled_text.T, video.T, pooled_audio.T], axis=0)
    xT = np.ascontiguousarray(
        xT.reshape(KT_X, 128, B).transpose(1, 0, 2)).astype(f16)
    xT = xT.reshape(128, KT_X * B)

    walls, biases = _fuse_weights(Wt, bt, Wgt, bgt, Wv, bv, Wgv, bgv,
                                  Wa, ba, Wga, bga)
    return [{"wall": walls[c], "xT": xT, "biases": biases[c]}
            for c in range(NCORES)]


def _postprocess(res):
    outs = []
    for e in EMBEDS:
        y = np.concatenate(
            [np.asarray(res.results[c][f"out_{e}"]) for c in range(NCORES)],
            axis=1).astype(np.float32)
        n = np.sqrt(np.sum(y * y, axis=1, keepdims=True))
        outs.append(y / np.maximum(n, 1e-12))
    return tuple(outs)


def kernel(text, video, audio_feats, Wt, bt, Wgt, bgt, Wv, bv, Wgv, bgv,
           Wa, ba, Wga, bga, nframes, raw_audio_len):
    from concourse.bass_utils import run_bass_kernel_spmd

    nc = _get_nc()
    in_maps = _prep_in_maps(text, video, audio_feats, Wt, bt, Wgt, bgt,
                            Wv, bv, Wgv, bgv, Wa, ba, Wga, bga,
                            nframes, raw_audio_len)
    res = run_bass_kernel_spmd(nc, in_maps, list(range(NCORES)))
    _STATE["last_results"] = res
    return _postprocess(res)
